# revision 1
# baseline (speedup 1.0000x reference)
"""Causal multi-head self-attention on 8 trn2 NeuronCores.

Sharding: core c = (batch b = c//2, head-group g = c%2). Each core handles one
batch element and 6 of the 12 heads: QKV projection for its 384 output dims,
causal attention for its 6 heads, and a partial output projection against the
matching 384 columns of o_proj. Host sums the two partials per batch.

Device-side layout (per core):
  xT  [768, 2048]   x transposed (host-side), d on partitions
  QT/KT pair tiles [128, 2048]: partitions = (head 2p | head 2p+1) x dk=64,
    free dim = sequence. Produced by out = wT.T @ xT matmuls.
  S^T tiles [k, q]: scores transposed, computed with head-pair row tiling
    (K=dk=64 per head, two heads in array rows 0-63 / 64-127).
  exp on ScalarE (PSUM -> SBUF, bf16), causal mask applied as a 0/1 multiply
    on the diagonal blocks only.
  V_aug [k, 65]: V for one head + ones column; A@V matmul then yields both
    O^T (rows 0..63) and the softmax denominator (row 64) in one chain.
  Normalization: denom reciprocal (DVE), broadcast across partitions via a
    K=1 ones matmul on PE, multiply on DVE -> OT tiles [c, s].
  Output projection: out = OT.T @ owT accumulated over the 3 c-blocks.
"""

import numpy as np
import ml_dtypes

B, S, D = 4, 2048, 768
H, DK = 12, 64
NCORES = 8
GH = 6        # heads per core
GO = GH * DK  # 384, per-core slice of the qkv output dim
NP = 3        # head pairs per core
NSB = S // 128   # 16 sequence blocks of 128
NJ = S // 512    # 4 q-chunks of 512

BF16 = ml_dtypes.bfloat16

_CACHE = {}


def _build_bass():
    import concourse.bass as bass  # noqa: F401
    import concourse.tile as tile
    from concourse import bacc, mybir
    from contextlib import ExitStack

    f32 = mybir.dt.float32
    bf16 = mybir.dt.bfloat16
    AF = mybir.ActivationFunctionType

    nc = bacc.Bacc("TRN2", target_bir_lowering=False, debug=False,
                   num_devices=NCORES)

    xT_d = nc.dram_tensor("xT", [D, S], bf16, kind="ExternalInput").ap()
    wqT_d = nc.dram_tensor("wqT", [D, GO], bf16, kind="ExternalInput").ap()
    wkT_d = nc.dram_tensor("wkT", [D, GO], bf16, kind="ExternalInput").ap()
    wvT_d = nc.dram_tensor("wvT", [D, GO], bf16, kind="ExternalInput").ap()
    owT_d = nc.dram_tensor("owT", [GO, D], bf16, kind="ExternalInput").ap()
    mk_d = nc.dram_tensor("mk", [128, 128], bf16, kind="ExternalInput").ap()
    sel_d = nc.dram_tensor("sel", [GH, GH * 64], f32, kind="ExternalInput").ap()
    part_d = nc.dram_tensor("part", [S, D], f32, kind="ExternalOutput").ap()

    ND = D // 128  # 6 d-blocks

    with tile.TileContext(nc) as tc, ExitStack() as ctx:
        pers = ctx.enter_context(tc.tile_pool(name="pers", bufs=1))

        # ---- persistent SBUF tiles -------------------------------------
        xT = [pers.tile([128, S], bf16, tag=f"xT{d}", name=f"xT{d}") for d in range(ND)]
        wq = [pers.tile([128, GO], bf16, tag=f"wq{d}", name=f"wq{d}") for d in range(ND)]
        wk = [pers.tile([128, GO], bf16, tag=f"wk{d}", name=f"wk{d}") for d in range(ND)]
        wv = [pers.tile([128, GO], bf16, tag=f"wv{d}", name=f"wv{d}") for d in range(ND)]
        ow = [pers.tile([128, D], bf16, tag=f"ow{c}", name=f"ow{c}") for c in range(NP)]
        mkt = pers.tile([128, 128], bf16, tag="mkt", name="mkt")
        QT = [pers.tile([128, S], bf16, tag=f"QT{p}", name=f"QT{p}") for p in range(NP)]
        KT = [pers.tile([128, S], bf16, tag=f"KT{p}", name=f"KT{p}") for p in range(NP)]
        OT = [pers.tile([128, S], bf16, tag=f"OT{p}", name=f"OT{p}") for p in range(NP)]
        vaug = [pers.tile([128, GH * 65], bf16, tag=f"va{kb}", name=f"va{kb}")
                for kb in range(NSB)]
        # selector matrix (host-built): sel_t[k, h*64+m] = (k == h); the
        # [6, 64] slice for head h is the lhsT of a K=6 matmul that
        # broadcasts reciprocal row h across 64 output partitions
        sel_t = pers.tile([GH, GH * 64], f32, tag="sel_t", name="sel_t")
        nc.sync.dma_start(sel_t[:], sel_d[:])

        # DMA priority: what the first score matmuls need lands first —
        # wq/wk, then the j=3 x columns, then the rest in reverse-j order.
        for d in range(ND):
            nc.sync.dma_start(wq[d][:], wqT_d[d * 128:(d + 1) * 128, :])
            nc.sync.dma_start(xT[d][:, 3 * 512:4 * 512],
                              xT_d[d * 128:(d + 1) * 128, 3 * 512:4 * 512])
        for d in range(ND):
            nc.gpsimd.dma_start(xT[d][:, 0:512], xT_d[d * 128:(d + 1) * 128, 0:512])
        for d in range(ND):
            nc.sync.dma_start(wk[d][:], wkT_d[d * 128:(d + 1) * 128, :])
        for d in range(ND):
            nc.gpsimd.dma_start(wv[d][:], wvT_d[d * 128:(d + 1) * 128, :])
        nc.sync.dma_start(mkt[:], mk_d[:])
        for jj in (1, 2):
            for d in range(ND):
                nc.sync.dma_start(xT[d][:, jj * 512:(jj + 1) * 512],
                                  xT_d[d * 128:(d + 1) * 128,
                                       jj * 512:(jj + 1) * 512])
        for c in range(NP):
            nc.gpsimd.dma_start(ow[c][:], owT_d[c * 128:(c + 1) * 128, :])
        # one unified PSUM pool: sp = scores strip (4 banks), u512 = shared
        # 1-bank slots for proj / AV / broadcast / output projection
        psp = ctx.enter_context(tc.tile_pool(name="psp", space="PSUM", bufs=2))
        pav = ctx.enter_context(tc.tile_pool(name="pav", space="PSUM", bufs=2))
        pu = ctx.enter_context(tc.tile_pool(name="pu", space="PSUM", bufs=2))
        expp = ctx.enter_context(tc.tile_pool(name="expp", bufs=6))
        sml = ctx.enter_context(tc.tile_pool(name="sml", bufs=3))
        stg = ctx.enter_context(tc.tile_pool(name="stg", bufs=4))

        def proj_qk(wsrc, dst, ob, j):
            ps = pu.tile([128, 512], f32, tag="u512", name="pjq")
            for d in range(ND):
                nc.tensor.matmul(ps[:], wsrc[d][:, ob * 128:(ob + 1) * 128],
                                 xT[d][:, j * 512:(j + 1) * 512],
                                 start=(d == 0), stop=(d == ND - 1))
            nc.vector.tensor_copy(dst[ob][:, j * 512:(j + 1) * 512], ps[:])

        def proj_v(sb):
            ps = pu.tile([128, GO], f32, tag="u512", name="pjv")
            for d in range(ND):
                nc.tensor.matmul(ps[:], xT[d][:, sb * 128:(sb + 1) * 128],
                                 wv[d][:], start=(d == 0), stop=(d == ND - 1))
            vt = vaug[sb]
            dst = vt[:].rearrange("p (h c) -> p h c", h=GH)[:, :, 0:64]
            nc.vector.tensor_copy(
                dst, ps[:].rearrange("p (h c) -> p h c", h=GH))
            nc.vector.memset(
                vt[:].rearrange("p (h c) -> p h c", h=GH)[:, :, 64:65], 1.0)

        # ---- attention, software-pipelined --------------------------------
        # One serial exp chain on ScalarE is the backbone. Per kb-group:
        # two score matmuls (head pair, row-tiled) -> exp -> mask; the AV
        # matmuls for group kb-1 run while exp(kb) drains, so PE's in-order
        # stream never waits on the chain. Filler thunks (V/QK projections,
        # normalization, output projection) are paced between groups.
        def attention_pair(j, p, dn, filler):
            # filler: list of (earliest_group, thunk); fired in list order
            # once the group index reaches the earliest_group mark.
            nkb = 4 * j + 4
            nfill = len(filler)
            av = [pav.tile([65, 512], f32, tag="av", name="av")
                  for _ in range(2)]
            pend = None
            fi = 0
            for kb in range(nkb):
                sp = psp.tile([128, 1024], f32, tag="sp", name="sp")
                ex = expp.tile([128, 1024], bf16, tag="ex", name="ex")
                v = kb - 4 * j  # >0: leading 128*v columns fully masked
                cut = 128 * max(v, 0)
                for hh in range(2):
                    nc.tensor.matmul(
                        sp[:, hh * 512 + cut:(hh + 1) * 512],
                        KT[p][hh * 64:(hh + 1) * 64,
                              kb * 128:(kb + 1) * 128],
                        QT[p][hh * 64:(hh + 1) * 64,
                              j * 512 + cut:(j + 1) * 512],
                        start=True, stop=True,
                        tile_position=(hh * 64, 0))
                if cut:
                    spv = sp[:].rearrange("q (s c) -> q s c", s=2)[:, :, cut:]
                    exv = ex[:].rearrange("q (s c) -> q s c", s=2)[:, :, cut:]
                    nc.scalar.activation(exv, spv, AF.Exp)
                else:
                    nc.scalar.activation(ex[:], sp[:], AF.Exp)
                if v >= 0:  # diagonal 128x128 block: causal triangle
                    for hh in range(2):
                        tri = ex[:, hh * 512 + cut:hh * 512 + cut + 128]
                        nc.vector.tensor_mul(tri, tri, mkt[:])
                if pend is not None:
                    pkb, pex, pcut = pend
                    for hh in range(2):
                        nc.tensor.matmul(
                            av[hh][:, pcut:],
                            vaug[pkb][:, (2 * p + hh) * 65:
                                      (2 * p + hh + 1) * 65],
                            pex[:, hh * 512 + pcut:(hh + 1) * 512],
                            start=(pkb == 0), stop=False)
                while fi < nfill and filler[fi][0] <= kb:
                    filler[fi][1]()
                    fi += 1
                pend = (kb, ex, cut)
            pkb, pex, pcut = pend
            for hh in range(2):
                nc.tensor.matmul(
                    av[hh][:, pcut:],
                    vaug[pkb][:, (2 * p + hh) * 65:(2 * p + hh + 1) * 65],
                    pex[:, hh * 512 + pcut:(hh + 1) * 512],
                    start=(pkb == 0), stop=True)
            while fi < nfill:
                filler[fi][1]()
                fi += 1
            for hh in range(2):
                h = 2 * p + hh
                nc.vector.tensor_copy(
                    OT[p][hh * 64:(hh + 1) * 64, j * 512:(j + 1) * 512],
                    av[hh][0:64, :])
                ds_ = sml.tile([1, 512], f32, tag="dstg", name="dstg",
                               bufs=6)
                nc.vector.tensor_copy(ds_[:], av[hh][64:65, :])
                nc.sync.dma_start(dn[h:h + 1, :], ds_[:])

        def norm_thunks(j, dn):
            rcp6 = sml.tile([6, 512], f32, tag="rcp6", name="rcp6")
            nc.vector.reciprocal(rcp6[:], dn[:])
            out = []

            def mk_bc(p):
                def f():
                    bc = pu.tile([128, 512], f32, tag="u512", name="bc")
                    for hh in range(2):
                        h = 2 * p + hh
                        nc.tensor.matmul(bc[hh * 64:(hh + 1) * 64, :],
                                         sel_t[:, h * 64:(h + 1) * 64],
                                         rcp6[:], start=True, stop=True,
                                         tile_position=(0, hh * 64))
                    nc.vector.tensor_mul(OT[p][:, j * 512:(j + 1) * 512],
                                         OT[p][:, j * 512:(j + 1) * 512],
                                         bc[:])
                return f

            def mk_op(sb):
                def f():
                    op1 = pu.tile([128, 512], f32, tag="u512", name="op1")
                    op2 = pu.tile([128, 256], f32, tag="u512", name="op2")
                    for cb in range(NP):
                        lhs = OT[cb][:, sb * 128:(sb + 1) * 128]
                        nc.tensor.matmul(op1[:], lhs, ow[cb][:, 0:512],
                                         start=(cb == 0), stop=(cb == NP - 1))
                        nc.tensor.matmul(op2[:], lhs, ow[cb][:, 512:768],
                                         start=(cb == 0), stop=(cb == NP - 1))
                    st = stg.tile([128, D], f32, tag="st", name="st")
                    nc.vector.tensor_copy(st[:, 0:512], op1[:])
                    nc.vector.tensor_copy(st[:, 512:768], op2[:])
                    nc.sync.dma_start(part_d[sb * 128:(sb + 1) * 128, :],
                                      st[:])
                return f

            for p in range(NP):
                out.append(mk_bc(p))
            for sb in range(4 * j, 4 * j + 4):
                out.append(mk_op(sb))
            return out

        proj_qk(wq, QT, 0, 3)
        proj_qk(wk, KT, 0, 0)

        def qthunk(ob, jj):
            return lambda: proj_qk(wq, QT, ob, jj)

        def kthunk(ob, jj):
            return lambda: proj_qk(wk, KT, ob, jj)

        def spread(thunks, ngroups):
            n = len(thunks)
            if n == 0:
                return []
            return [(min(i * ngroups // n, ngroups - 1), t)
                    for i, t in enumerate(thunks)]

        # era index e = (3-j)*3 + p, in attention call order. Fillers are
        # (earliest_group, thunk); producers always land one era (or more)
        # ahead of their consumers.
        fills = [[] for _ in range(12)]
        # era 0 (j=3, p0, 16 groups): V per group (AV of group kb needs
        # vaug[kb] emitted by group kb), K0 chunks for the ascending kb loop,
        # then pair-1 K/Q late.
        f0 = [(0, lambda sb=sb: proj_v(sb)) for sb in range(NSB)]
        f0[1:1] = [(0, kthunk(0, 1))]
        f0[3:3] = [(1, kthunk(0, 2))]
        f0[5:5] = [(2, kthunk(0, 3))]
        f0 += [(9, kthunk(1, 0)), (10, kthunk(1, 1)), (11, kthunk(1, 2)),
               (12, kthunk(1, 3)), (13, qthunk(1, 3))]
        fills[0] = f0
        fills[1] = spread([kthunk(2, 0), kthunk(2, 1), kthunk(2, 2),
                           kthunk(2, 3), qthunk(2, 3)], 16)
        fills[2] = spread([qthunk(0, 2)], 16)
        qlate = {3: qthunk(1, 2), 4: qthunk(2, 2), 5: qthunk(0, 1),
                 6: qthunk(1, 1), 7: qthunk(2, 1), 8: qthunk(0, 0),
                 9: qthunk(1, 0), 10: qthunk(2, 0)}

        prev = None
        era = 0
        for idx, j in enumerate(reversed(range(NJ))):
            ng = 4 * j + 4
            dn = sml.tile([6, 512], f32, tag="dn", name="dn")
            if idx > 0:
                nt = norm_thunks(prev[0], prev[1])
                fills[era] = spread(nt[:3], ng) + fills[era]
                fills[era + 1] = spread(nt[3:5], ng) + fills[era + 1]
                fills[era + 2] = spread(nt[5:], ng) + fills[era + 2]
            for p in range(NP):
                f = fills[era]
                if era in qlate:
                    f = f + [(ng - 2, qlate[era])]
                f.sort(key=lambda x: x[0])
                attention_pair(j, p, dn, f)
                era += 1
            prev = (j, dn)
        for t in norm_thunks(prev[0], prev[1]):
            t()

    nc.compile()
    return nc


def _prep_in_maps(in_features, qkv_proj_weight, o_proj_weight):
    """Per-core input dict (host-side shard + transpose + cast)."""
    # causal 0/1 triangle for diagonal 128x128 blocks
    r = np.arange(128)[:, None]
    c = np.arange(128)[None, :]
    mk = (r <= c).astype(np.float32).astype(BF16)
    sel = np.zeros((GH, GH * 64), np.float32)
    for h in range(GH):
        sel[h, h * 64:(h + 1) * 64] = 1.0

    scale = 1.0 / np.sqrt(np.float32(DK))
    in_maps = []
    for core in range(NCORES):
        b, g = core // 2, core % 2
        sl = slice(g * GO, (g + 1) * GO)
        xT = np.ascontiguousarray(in_features[b].T).astype(BF16)
        wqT = np.ascontiguousarray((qkv_proj_weight[0][sl, :] * scale).T
                                   ).astype(BF16)
        wkT = np.ascontiguousarray(qkv_proj_weight[1][sl, :].T).astype(BF16)
        wvT = np.ascontiguousarray(qkv_proj_weight[2][sl, :].T).astype(BF16)
        owT = np.ascontiguousarray(o_proj_weight[:, sl].T).astype(BF16)
        in_maps.append({"xT": xT, "wqT": wqT, "wkT": wkT, "wvT": wvT,
                        "owT": owT, "mk": mk, "sel": sel})
    return in_maps


def _get_runner():
    """Persistent sharded-jit runner over the 8 NeuronCores.

    Mirrors bass_utils.run_bass_kernel_spmd's axon path
    (bass2jax.run_bass_via_pjrt), but keeps the jitted executable cached
    across calls and skips install_neuronx_cc_hook: under axon the
    bass_exec custom-call is compiled terminal-side, and the client-side
    hook rejects the SPMD-partitioned HLO.
    """
    if "runner" in _CACHE:
        return _CACHE["runner"]

    import jax
    from concourse import mybir
    from concourse.bass2jax import _bass_exec_p, partition_id_tensor
    from jax.sharding import Mesh, PartitionSpec
    from jax.experimental.shard_map import shard_map

    nc = _build_bass()

    partition_name = (nc.partition_id_tensor.name
                      if nc.partition_id_tensor else None)
    in_names, out_names, out_avals, zero_outs = [], [], [], []
    for alloc in nc.m.functions[0].allocations:
        if not isinstance(alloc, mybir.MemoryLocationSet):
            continue
        name = alloc.memorylocations[0].name
        if alloc.kind == "ExternalInput":
            if name != partition_name:
                in_names.append(name)
        elif alloc.kind == "ExternalOutput":
            out_names.append(name)
            shape = tuple(alloc.tensor_shape)
            dtype = mybir.dt.np(alloc.dtype)
            out_avals.append(jax.core.ShapedArray(shape, dtype))
            zero_outs.append(np.zeros(shape, dtype))
    n_params = len(in_names)
    n_outs = len(out_avals)
    all_in = list(in_names) + out_names + (
        [partition_name] if partition_name else [])

    def _body(*args):
        operands = list(args)
        if partition_name is not None:
            operands.append(partition_id_tensor())
        return tuple(_bass_exec_p.bind(
            *operands,
            out_avals=tuple(out_avals),
            in_names=tuple(all_in),
            out_names=tuple(out_names),
            lowering_input_output_aliases=(),
            sim_require_finite=True, sim_require_nnan=True, nc=nc))

    devices = jax.devices()[:NCORES]
    mesh = Mesh(np.asarray(devices), ("core",))
    fn = jax.jit(
        shard_map(_body, mesh=mesh,
                  in_specs=(PartitionSpec("core"),) * (n_params + n_outs),
                  out_specs=(PartitionSpec("core"),) * n_outs,
                  check_rep=False),
        donate_argnums=tuple(range(n_params, n_params + n_outs)),
        keep_unused=True)

    def run(in_maps):
        per_core = [[np.asarray(m[n]) for n in in_names] for m in in_maps]
        concat_in = [np.concatenate([per_core[c][i] for c in range(NCORES)],
                                    axis=0) for i in range(n_params)]
        concat_zeros = [np.zeros((NCORES * z.shape[0], *z.shape[1:]), z.dtype)
                        for z in zero_outs]
        out_arrs = fn(*concat_in, *concat_zeros)
        return np.asarray(out_arrs[out_names.index("part")]).reshape(
            NCORES, S, D)

    _CACHE["nc"] = nc
    _CACHE["runner"] = run
    return run


def kernel(in_features, qkv_proj_weight, o_proj_weight):
    run = _get_runner()
    in_maps = _prep_in_maps(np.asarray(in_features, np.float32),
                            np.asarray(qkv_proj_weight, np.float32),
                            np.asarray(o_proj_weight, np.float32))
    parts = run(in_maps)
    out = np.empty((B, S, D), np.float32)
    for b in range(B):
        out[b] = parts[2 * b] + parts[2 * b + 1]
    return out



# revision 12
# speedup vs baseline: 1.2370x; 1.2370x over previous
"""Causal multi-head self-attention on 8 trn2 NeuronCores.

Sharding: core c = (batch b = c//2, head-group g = c%2). Each core handles one
batch element and 6 of the 12 heads: QKV projection for its 384 output dims,
causal attention for its 6 heads, and a partial output projection against the
matching 384 columns of o_proj. Host sums the two partials per batch.

Device-side layout (per core):
  xT  [768, 2048]   x transposed (host-side), d on partitions
  QT/KT pair tiles [128, 2048]: partitions = (head 2p | head 2p+1) x dk=64,
    free dim = sequence. Produced by out = wT.T @ xT matmuls.
  S^T tiles [k, q]: scores transposed, computed with head-pair row tiling
    (K=dk=64 per head, two heads in array rows 0-63 / 64-127).
  exp on ScalarE (PSUM -> SBUF) emits fp8e4 with bias=-3 (the e^-3 factor
    cancels in softmax normalization; keeps exp values in fp8e4 range).
  A@V runs in fp8 DoubleRow perf mode: two adjacent 128-key blocks form the
    two DoubleRow sub-matrices ([128, 2, .] APs), contracting 256 keys per
    instruction at 0.5 cycles/row -- 4x fewer PE cycles than bf16.
  vaug8 [128, 2, 6*65] fp8 per block-pair: V for 6 heads + ones column; the
    A@V matmul chain yields both O^T and the softmax denominator.
  Normalization: denom reciprocal (DVE), broadcast across partitions via a
    K=6 selector matmul on PE, multiply on DVE -> OT tiles [c, s].
  Output projection: out = OT.T @ owT accumulated over the 3 c-blocks.
"""

import numpy as np
import ml_dtypes

B, S, D = 4, 2048, 768
H, DK = 12, 64
NCORES = 8
GH = 6        # heads per core
GO = GH * DK  # 384, per-core slice of the qkv output dim
NP = 3        # head pairs per core
NSB = S // 128   # 16 sequence blocks of 128
NJ = S // 512    # 4 q-chunks of 512

BF16 = ml_dtypes.bfloat16
F8 = ml_dtypes.float8_e4m3

_CACHE = {}


def _build_bass():
    import concourse.bass as bass  # noqa: F401
    import concourse.tile as tile
    from concourse import bacc, mybir
    from contextlib import ExitStack

    f32 = mybir.dt.float32
    bf16 = mybir.dt.bfloat16
    f8 = mybir.dt.float8e4
    AF = mybir.ActivationFunctionType
    DR = mybir.MatmulPerfMode.DoubleRow

    nc = bacc.Bacc("TRN2", target_bir_lowering=False, debug=False,
                   num_devices=NCORES)

    xT_d = nc.dram_tensor("xT", [D, S], bf16, kind="ExternalInput").ap()
    wqT_d = nc.dram_tensor("wqT", [D, GO], bf16, kind="ExternalInput").ap()
    wkT_d = nc.dram_tensor("wkT", [D, GO], bf16, kind="ExternalInput").ap()
    wvT_d = nc.dram_tensor("wvT", [D, GO], bf16, kind="ExternalInput").ap()
    owT_d = nc.dram_tensor("owT", [GO, D], bf16, kind="ExternalInput").ap()
    mk_d = nc.dram_tensor("mk", [128, 128], bf16, kind="ExternalInput").ap()
    mk2_d = nc.dram_tensor("mk2", [128, 256], bf16, kind="ExternalInput").ap()
    sel_d = nc.dram_tensor("sel", [GH, GH * 64], f32, kind="ExternalInput").ap()
    part_d = nc.dram_tensor("part", [S, D], f32, kind="ExternalOutput").ap()

    ND = D // 128  # 6 d-blocks

    with tile.TileContext(nc) as tc, ExitStack() as ctx:
        pers = ctx.enter_context(tc.tile_pool(name="pers", bufs=1))

        # ---- persistent SBUF tiles -------------------------------------
        xT = [pers.tile([128, S], bf16, tag=f"xT{d}", name=f"xT{d}") for d in range(ND)]
        wq = [pers.tile([128, GO], bf16, tag=f"wq{d}", name=f"wq{d}") for d in range(ND)]
        wk = [pers.tile([128, GO], bf16, tag=f"wk{d}", name=f"wk{d}") for d in range(ND)]
        wv = [pers.tile([128, GO], bf16, tag=f"wv{d}", name=f"wv{d}") for d in range(ND)]
        ow = [pers.tile([128, D], bf16, tag=f"ow{c}", name=f"ow{c}") for c in range(NP)]
        mkt = pers.tile([128, 128], bf16, tag="mkt", name="mkt")
        mk2t = pers.tile([128, 256], bf16, tag="mk2t", name="mk2t")
        QT = [pers.tile([128, S], bf16, tag=f"QT{p}", name=f"QT{p}") for p in range(NP)]
        KT = [pers.tile([128, S], bf16, tag=f"KT{p}", name=f"KT{p}") for p in range(NP)]
        OT = [pers.tile([128, S], bf16, tag=f"OT{p}", name=f"OT{p}") for p in range(NP)]
        # fp8 V+ones per key-block pair: [k=128, i=2, x=400] where
        # x = [h=6, c=65] + 10 pad bytes -- the DoubleRow ldweights ISA
        # check requires the sub-matrix stride to be 16-byte aligned
        vaug8 = [pers.tile([128, 800], f8, tag=f"va{pb}", name=f"va{pb}")
                 for pb in range(NSB // 2)]
        # bf16 V+ones for blocks 0-3 (the j=0 era: q<512 stays bf16 since
        # fp8 A-noise there fails the max-err budget -- small denominators)
        vaugb = [pers.tile([128, GH * 65], bf16, tag=f"vb{sb}", name=f"vb{sb}")
                 for sb in range(4)]
        # selector matrix (host-built): sel_t[k, h*64+m] = (k == h); the
        # [6, 64] slice for head h is the lhsT of a K=6 matmul that
        # broadcasts reciprocal row h across 64 output partitions
        sel_t = pers.tile([GH, GH * 64], f32, tag="sel_t", name="sel_t")
        bneg3 = pers.tile([128, 1], f32, tag="bneg3", name="bneg3")
        nc.vector.memset(bneg3[:], -3.0)

        # DMA priority, spread across engine queues: what the first score
        # matmuls need lands first (wq + j=3 x columns on sync; wk on
        # vector; j=0 x columns on gpsimd; the rest follows).
        for d in range(ND):
            nc.sync.dma_start(wq[d][:], wqT_d[d * 128:(d + 1) * 128, :])
            nc.sync.dma_start(xT[d][:, 3 * 512:4 * 512],
                              xT_d[d * 128:(d + 1) * 128, 3 * 512:4 * 512])
        for d in range(ND):
            nc.scalar.dma_start(wk[d][:], wkT_d[d * 128:(d + 1) * 128, :])
        for d in range(ND):
            nc.gpsimd.dma_start(xT[d][:, 0:512], xT_d[d * 128:(d + 1) * 128, 0:512])
        for d in range(ND):
            nc.gpsimd.dma_start(wv[d][:], wvT_d[d * 128:(d + 1) * 128, :])
        nc.scalar.dma_start(mkt[:], mk_d[:])
        nc.scalar.dma_start(mk2t[:], mk2_d[:])
        nc.scalar.dma_start(sel_t[:], sel_d[:])
        for jj in (1, 2):
            for d in range(ND):
                nc.sync.dma_start(xT[d][:, jj * 512:(jj + 1) * 512],
                                  xT_d[d * 128:(d + 1) * 128,
                                       jj * 512:(jj + 1) * 512])
        for c in range(NP):
            nc.gpsimd.dma_start(ow[c][:], owT_d[c * 128:(c + 1) * 128, :])
        # PSUM budget: sp strips 2x2 banks, av 2x1, u512 2x1 = 8 banks
        psp = ctx.enter_context(tc.tile_pool(name="psp", space="PSUM", bufs=2))
        pav = ctx.enter_context(tc.tile_pool(name="pav", space="PSUM", bufs=2))
        pu = ctx.enter_context(tc.tile_pool(name="pu", space="PSUM", bufs=2))
        expp = ctx.enter_context(tc.tile_pool(name="expp", bufs=6))
        sml = ctx.enter_context(tc.tile_pool(name="sml", bufs=3))
        stg = ctx.enter_context(tc.tile_pool(name="stg", bufs=4))

        def proj_qk(wsrc, dst, ob, j):
            ps = pu.tile([128, 512], f32, tag="u512", name="pjq")
            for d in range(ND):
                nc.tensor.matmul(ps[:], wsrc[d][:, ob * 128:(ob + 1) * 128],
                                 xT[d][:, j * 512:(j + 1) * 512],
                                 start=(d == 0), stop=(d == ND - 1))
            nc.vector.tensor_copy(dst[ob][:, j * 512:(j + 1) * 512], ps[:])

        def proj_v(sb):
            ps = pu.tile([128, GO], f32, tag="u512", name="pjv")
            for d in range(ND):
                nc.tensor.matmul(ps[:], xT[d][:, sb * 128:(sb + 1) * 128],
                                 wv[d][:], start=(d == 0), stop=(d == ND - 1))
            vv = vaug8[sb // 2][:].rearrange("k (i x) -> k i x", i=2)[
                :, sb % 2, 0:GH * 65].rearrange("k (h c) -> k h c", h=GH)
            nc.vector.tensor_copy(
                vv[:, :, 0:64], ps[:].rearrange("k (h c) -> k h c", h=GH))
            nc.vector.memset(vv[:, :, 64:65], 1.0)
            if sb < 4:  # bf16 copy for the j=0 era
                vb = vaugb[sb][:].rearrange("k (h c) -> k h c", h=GH)
                nc.vector.tensor_copy(
                    vb[:, :, 0:64], ps[:].rearrange("k (h c) -> k h c", h=GH))
                nc.vector.memset(vb[:, :, 64:65], 1.0)

        # ---- attention, software-pipelined --------------------------------
        # One serial exp chain on ScalarE is the backbone. Per key-block
        # PAIR: four score matmuls (2 blocks x head pair, row-tiled) -> two
        # exps (fp8 out) -> masks; the DoubleRow AV matmuls for pair-1 run
        # while exp drains. Filler thunks (V/QK projections, normalization,
        # output projection) are paced between pairs.
        def attention_pair(j, p, dn, filler):
            nfill = len(filler)
            av = [pav.tile([65, 512], f32, tag="av", name="av")
                  for _ in range(2)]
            pend = None
            fi = 0
            if j == 0:
                # bf16 per-block path for q<512 (fp8 A-noise too large there)
                for kb in range(4):
                    sp = psp.tile([128, 1024], f32, tag="sp", name="sp")
                    ex = expp.tile([128, 1024], bf16, tag="exb", name="exb",
                                   bufs=3)
                    cut = 128 * kb
                    for hh in range(2):
                        nc.tensor.matmul(
                            sp[:, hh * 512 + cut:(hh + 1) * 512],
                            KT[p][hh * 64:(hh + 1) * 64,
                                  kb * 128:(kb + 1) * 128],
                            QT[p][hh * 64:(hh + 1) * 64, cut:512],
                            start=True, stop=True,
                            tile_position=(hh * 64, 0))
                    if cut:
                        spv = sp[:].rearrange("q (s c) -> q s c",
                                              s=2)[:, :, cut:]
                        exv = ex[:].rearrange("q (s c) -> q s c",
                                              s=2)[:, :, cut:]
                        nc.scalar.activation(exv, spv, AF.Exp)
                    else:
                        nc.scalar.activation(ex[:], sp[:], AF.Exp)
                    for hh in range(2):
                        tri = ex[:, hh * 512 + cut:hh * 512 + cut + 128]
                        nc.vector.tensor_mul(tri, tri, mkt[:])
                    if pend is not None:
                        pkb, pex, pcut = pend
                        for hh in range(2):
                            nc.tensor.matmul(
                                av[hh][:, pcut:],
                                vaugb[pkb][:, (2 * p + hh) * 65:
                                           (2 * p + hh + 1) * 65],
                                pex[:, hh * 512 + pcut:(hh + 1) * 512],
                                start=(pkb == 0), stop=False)
                    while fi < nfill and filler[fi][0] <= kb:
                        filler[fi][1]()
                        fi += 1
                    pend = (kb, ex, cut)
                pkb, pex, pcut = pend
                for hh in range(2):
                    nc.tensor.matmul(
                        av[hh][:, pcut:],
                        vaugb[pkb][:, (2 * p + hh) * 65:
                                   (2 * p + hh + 1) * 65],
                        pex[:, hh * 512 + pcut:(hh + 1) * 512],
                        start=(pkb == 0), stop=True)
            else:
                npp = 2 * j + 2
                for pp in range(npp):
                    exf = expp.tile([128, 2048], f8, tag="ex", name="ex")
                    exv = exf[:].rearrange("k (i h q) -> k i h q", i=2, h=2)
                    cut = 256 if pp == 2 * j + 1 else 0
                    for i in range(2):
                        kb = 2 * pp + i
                        sp = psp.tile([128, 1024], f32, tag="sp", name="sp")
                        for hh in range(2):
                            nc.tensor.matmul(
                                sp[:, hh * 512 + cut:(hh + 1) * 512],
                                KT[p][hh * 64:(hh + 1) * 64,
                                      kb * 128:(kb + 1) * 128],
                                QT[p][hh * 64:(hh + 1) * 64,
                                      j * 512 + cut:(j + 1) * 512],
                                start=True, stop=True,
                                tile_position=(hh * 64, 0))
                        spv = sp[:].rearrange("k (h q) -> k h q",
                                              h=2)[:, :, cut:]
                        nc.scalar.activation(exv[:, i, :, cut:], spv, AF.Exp,
                                             bias=bneg3[:])
                    if pp >= 2 * j:  # diagonal pair: causal masks
                        for hh in range(2):
                            t0 = exv[:, 0, hh, cut:cut + 128]
                            nc.vector.tensor_mul(t0, t0, mkt[:])
                            t1 = exv[:, 1, hh, cut:cut + 256]
                            nc.vector.tensor_mul(t1, t1, mk2t[:])
                    if pend is not None:
                        ppk, pex, pcut = pend
                        for hh in range(2):
                            nc.tensor.matmul(
                                av[hh][:, pcut:],
                                vaug8[ppk][:].rearrange(
                                    "k (i x) -> k i x", i=2
                                )[:, :, (2 * p + hh) * 65:
                                  (2 * p + hh + 1) * 65],
                                pex[:, :, hh, pcut:],
                                start=(ppk == 0), stop=False, perf_mode=DR)
                    while fi < nfill and filler[fi][0] <= pp:
                        filler[fi][1]()
                        fi += 1
                    pend = (pp, exv, cut)
                ppk, pex, pcut = pend
                for hh in range(2):
                    nc.tensor.matmul(
                        av[hh][:, pcut:],
                        vaug8[ppk][:].rearrange(
                            "k (i x) -> k i x", i=2
                        )[:, :, (2 * p + hh) * 65:(2 * p + hh + 1) * 65],
                        pex[:, :, hh, pcut:],
                        start=(ppk == 0), stop=True, perf_mode=DR)
            while fi < nfill:
                filler[fi][1]()
                fi += 1
            for hh in range(2):
                h = 2 * p + hh
                nc.vector.tensor_copy(
                    OT[p][hh * 64:(hh + 1) * 64, j * 512:(j + 1) * 512],
                    av[hh][0:64, :])
                ds_ = sml.tile([1, 512], f32, tag="dstg", name="dstg",
                               bufs=6)
                nc.vector.tensor_copy(ds_[:], av[hh][64:65, :])
                nc.gpsimd.dma_start(dn[h:h + 1, :], ds_[:])

        def norm_thunks(j, dn):
            rcp6 = sml.tile([6, 512], f32, tag="rcp6", name="rcp6")
            nc.vector.reciprocal(rcp6[:], dn[:])
            out = []

            def mk_bc(p):
                def f():
                    bc = pu.tile([128, 512], f32, tag="u512", name="bc")
                    for hh in range(2):
                        h = 2 * p + hh
                        nc.tensor.matmul(bc[hh * 64:(hh + 1) * 64, :],
                                         sel_t[:, h * 64:(h + 1) * 64],
                                         rcp6[:], start=True, stop=True,
                                         tile_position=(0, hh * 64))
                    nc.vector.tensor_mul(OT[p][:, j * 512:(j + 1) * 512],
                                         OT[p][:, j * 512:(j + 1) * 512],
                                         bc[:])
                return f

            def mk_op(sb):
                def f():
                    op1 = pu.tile([128, 512], f32, tag="u512", name="op1")
                    op2 = pu.tile([128, 256], f32, tag="u512", name="op2")
                    for cb in range(NP):
                        lhs = OT[cb][:, sb * 128:(sb + 1) * 128]
                        nc.tensor.matmul(op1[:], lhs, ow[cb][:, 0:512],
                                         start=(cb == 0), stop=(cb == NP - 1))
                        nc.tensor.matmul(op2[:], lhs, ow[cb][:, 512:768],
                                         start=(cb == 0), stop=(cb == NP - 1))
                    st = stg.tile([128, D], f32, tag="st", name="st")
                    nc.vector.tensor_copy(st[:, 0:512], op1[:])
                    nc.vector.tensor_copy(st[:, 512:768], op2[:])
                    nc.sync.dma_start(part_d[sb * 128:(sb + 1) * 128, :],
                                      st[:])
                return f

            for p in range(NP):
                out.append(mk_bc(p))
            for sb in range(4 * j, 4 * j + 4):
                out.append(mk_op(sb))
            return out

        proj_qk(wq, QT, 0, 3)
        proj_qk(wk, KT, 0, 0)

        def qthunk(ob, jj):
            return lambda: proj_qk(wq, QT, ob, jj)

        def kthunk(ob, jj):
            return lambda: proj_qk(wk, KT, ob, jj)

        def spread(thunks, ngroups):
            n = len(thunks)
            if n == 0:
                return []
            return [(min(i * ngroups // n, ngroups - 1), t)
                    for i, t in enumerate(thunks)]

        # era index e = (3-j)*3 + p, in attention call order. Fillers are
        # (earliest_group, thunk); producers always land one era (or more)
        # ahead of their consumers. Groups are key-block PAIRS now
        # (ng = 2j+2 per era).
        fills = [[] for _ in range(12)]
        # era 0 (j=3, p0, 8 pair-groups): V per pair (AV of pair pp needs
        # vaug8[pp] emitted by pair pp), K0 chunks for the ascending loop,
        # then pair-1 K/Q late.
        f0 = [(max(0, sb // 2 - 1), lambda sb=sb: proj_v(sb))
              for sb in range(NSB)]
        f0[1:1] = [(0, kthunk(0, 1))]
        f0[3:3] = [(1, kthunk(0, 2))]
        f0[5:5] = [(3, kthunk(0, 3))]
        f0 += [(4, kthunk(1, 0)), (5, kthunk(1, 1)), (5, kthunk(1, 2)),
               (6, kthunk(1, 3)), (6, qthunk(1, 3))]
        fills[0] = f0
        fills[1] = spread([kthunk(2, 0), kthunk(2, 1), kthunk(2, 2),
                           kthunk(2, 3), qthunk(2, 3)], 8)
        fills[2] = spread([qthunk(0, 2)], 8)
        qlate = {3: qthunk(1, 2), 4: qthunk(2, 2), 5: qthunk(0, 1),
                 6: qthunk(1, 1), 7: qthunk(2, 1), 8: qthunk(0, 0),
                 9: qthunk(1, 0), 10: qthunk(2, 0)}

        prev = None
        era = 0
        for idx, j in enumerate(reversed(range(NJ))):
            ng = 4 if j == 0 else 2 * j + 2
            dn = sml.tile([6, 512], f32, tag="dn", name="dn")
            if idx > 0:
                nt = norm_thunks(prev[0], prev[1])
                fills[era] = spread(nt[:3], ng) + fills[era]
                fills[era + 1] = spread(nt[3:5], ng) + fills[era + 1]
                fills[era + 2] = spread(nt[5:], ng) + fills[era + 2]
            for p in range(NP):
                f = fills[era]
                if era in qlate:
                    f = f + [(ng - 2, qlate[era])]
                f.sort(key=lambda x: x[0])
                attention_pair(j, p, dn, f)
                era += 1
            prev = (j, dn)
        for t in norm_thunks(prev[0], prev[1]):
            t()

    nc.compile()
    return nc


def _prep_in_maps(in_features, qkv_proj_weight, o_proj_weight):
    """Per-core input dict (host-side shard + transpose + cast)."""
    # causal 0/1 triangle for diagonal 128x128 blocks (rows=k, cols=q)
    r = np.arange(128)[:, None]
    c = np.arange(128)[None, :]
    tri = (r <= c).astype(np.float32)
    mk = tri.astype(BF16)
    # second block of a diagonal pair: first 128 q-cols fully masked
    mk2 = np.concatenate([np.zeros((128, 128), np.float32), tri],
                         axis=1).astype(BF16)
    sel = np.zeros((GH, GH * 64), np.float32)
    for h in range(GH):
        sel[h, h * 64:(h + 1) * 64] = 1.0

    scale = 1.0 / np.sqrt(np.float32(DK))
    in_maps = []
    for core in range(NCORES):
        b, g = core // 2, core % 2
        sl = slice(g * GO, (g + 1) * GO)
        xT = np.ascontiguousarray(in_features[b].T).astype(BF16)
        wqT = np.ascontiguousarray((qkv_proj_weight[0][sl, :] * scale).T
                                   ).astype(BF16)
        wkT = np.ascontiguousarray(qkv_proj_weight[1][sl, :].T).astype(BF16)
        wvT = np.ascontiguousarray(qkv_proj_weight[2][sl, :].T).astype(BF16)
        owT = np.ascontiguousarray(o_proj_weight[:, sl].T).astype(BF16)
        in_maps.append({"xT": xT, "wqT": wqT, "wkT": wkT, "wvT": wvT,
                        "owT": owT, "mk": mk, "mk2": mk2, "sel": sel})
    return in_maps


def _get_runner():
    """Persistent sharded-jit runner over the 8 NeuronCores.

    Mirrors bass_utils.run_bass_kernel_spmd's axon path
    (bass2jax.run_bass_via_pjrt), but keeps the jitted executable cached
    across calls and skips install_neuronx_cc_hook: under axon the
    bass_exec custom-call is compiled terminal-side, and the client-side
    hook rejects the SPMD-partitioned HLO.
    """
    if "runner" in _CACHE:
        return _CACHE["runner"]

    import jax
    from concourse import mybir
    from concourse.bass2jax import _bass_exec_p, partition_id_tensor
    from jax.sharding import Mesh, PartitionSpec
    from jax.experimental.shard_map import shard_map

    nc = _build_bass()

    partition_name = (nc.partition_id_tensor.name
                      if nc.partition_id_tensor else None)
    in_names, out_names, out_avals, zero_outs = [], [], [], []
    for alloc in nc.m.functions[0].allocations:
        if not isinstance(alloc, mybir.MemoryLocationSet):
            continue
        name = alloc.memorylocations[0].name
        if alloc.kind == "ExternalInput":
            if name != partition_name:
                in_names.append(name)
        elif alloc.kind == "ExternalOutput":
            out_names.append(name)
            shape = tuple(alloc.tensor_shape)
            dtype = mybir.dt.np(alloc.dtype)
            out_avals.append(jax.core.ShapedArray(shape, dtype))
            zero_outs.append(np.zeros(shape, dtype))
    n_params = len(in_names)
    n_outs = len(out_avals)
    all_in = list(in_names) + out_names + (
        [partition_name] if partition_name else [])

    def _body(*args):
        operands = list(args)
        if partition_name is not None:
            operands.append(partition_id_tensor())
        return tuple(_bass_exec_p.bind(
            *operands,
            out_avals=tuple(out_avals),
            in_names=tuple(all_in),
            out_names=tuple(out_names),
            lowering_input_output_aliases=(),
            sim_require_finite=True, sim_require_nnan=True, nc=nc))

    devices = jax.devices()[:NCORES]
    mesh = Mesh(np.asarray(devices), ("core",))
    fn = jax.jit(
        shard_map(_body, mesh=mesh,
                  in_specs=(PartitionSpec("core"),) * (n_params + n_outs),
                  out_specs=(PartitionSpec("core"),) * n_outs,
                  check_rep=False),
        donate_argnums=tuple(range(n_params, n_params + n_outs)),
        keep_unused=True)

    def run(in_maps):
        per_core = [[np.asarray(m[n]) for n in in_names] for m in in_maps]
        concat_in = [np.concatenate([per_core[c][i] for c in range(NCORES)],
                                    axis=0) for i in range(n_params)]
        concat_zeros = [np.zeros((NCORES * z.shape[0], *z.shape[1:]), z.dtype)
                        for z in zero_outs]
        out_arrs = fn(*concat_in, *concat_zeros)
        return np.asarray(out_arrs[out_names.index("part")]).reshape(
            NCORES, S, D)

    _CACHE["nc"] = nc
    _CACHE["runner"] = run
    return run


def kernel(in_features, qkv_proj_weight, o_proj_weight):
    run = _get_runner()
    in_maps = _prep_in_maps(np.asarray(in_features, np.float32),
                            np.asarray(qkv_proj_weight, np.float32),
                            np.asarray(o_proj_weight, np.float32))
    parts = run(in_maps)
    out = np.empty((B, S, D), np.float32)
    for b in range(B):
        out[b] = parts[2 * b] + parts[2 * b + 1]
    return out
